# revision 7
# baseline (speedup 1.0000x reference)
"""Expert-choice MoE kernel for 8 Trainium2 NeuronCores (expert-parallel).

Decomposition (core e handles expert e):
  - router logits x . emb_e computed in fp32 on PE; top-8 token indices per
    batch row via DVE max8/max_index; token gather via indirect DMA.
  - sum_weights GEMM1 column-sharded (each core owns 1536 columns of sw_w1);
    the tiny (8,64) partial logits are AllReduced, softmaxed locally.
  - expert MLP: GEMM1 (w1) in bf16; GEMM2 (w2) weights streamed as fp8e3m4
    scaled x128 (descale folded into the combine weights `wes`).
  - er * w[:, e] contributions AllReduced in bf16, in 3 column chunks of
    1024 that pipeline with the w2 stream; each arrived ws chunk feeds the
    (column-sharded) head GEMM1 accumulation immediately.
  - classification head sharded: GEMM1 column-shard (384 cols of ch_w1),
    GEMM2 contraction-shard (384 rows of ch_w2); per-core (64,1000) partials
    are summed on the host (+ ch_b2).

DMA ring assignment: sync(HWDGE/SP) ring carries ONLY the big weight
stream (sw1 -> w1 -> w2) in consumption order, packed into large
contiguous chunks (1.5MB sw / 1.5MB w1 / 1MB w2). Activations, inputs,
ch1/ch2 and collective staging ride the scalar(Act) ring; gathers and
collective triggers ride gpsimd (SWDGE).

PSUM plan (8 banks): tag "pm" bufs=6 + tag "pt" bufs=2 (transposes +
router). The "pm" ring allocation order is load-bearing (ring reuse must
only ever land on a dead tile): pms x3, pz, pme x6, pme2_j0 x2, pmh,
pme2_j1 x2, pme2_j2 x2, pmo x2 (single er-AR after all GEMM2 chunks).
"""

import numpy as np
import ml_dtypes

import concourse.bass as bass
from concourse import bacc
import concourse.mybir as mybir
import concourse.tile as tile
from concourse.bass import ts, ds
from concourse.bass_utils import run_bass_kernel_spmd
from concourse.masks import make_identity

B, N, D, E, K, C = 64, 32, 384, 8, 8, 1000
KD, ND = K * D, N * D          # 3072, 12288
P = 128
NTOK = B * N                   # 2048
SWC = ND // E                  # 1536 sum-weights columns per core
CH1C = KD // E                 # 384 head-GEMM1 columns per core
KCE = KD // P                  # 24 k-chunks, expert GEMMs
KCS = ND // P                  # 96 k-chunks, sum-weights GEMM1
KCH = SWC // P                 # 12 k-chunks, z GEMM
NCORES = 8

SWPACK = 4                     # k-chunks per sw1 DMA (1.5MB)
NSW = KCS // SWPACK            # 24 sw tiles
W1PACK = 2                     # k-chunks per w1 DMA (1.5MB)
NW1 = KCE // W1PACK            # 12 w1 tiles
NJ = 3                         # er/AllReduce column chunks
JW = KD // NJ                  # 1024 columns per chunk
W2SUB = 3                      # sub-DMAs per w2 column chunk
W2K = KCE // W2SUB             # 8 k-chunks per w2 sub-DMA

W2_FP8 = True                  # stream w2 as fp8e3m4 (scale 128)
W2_SCALE = 128.0

F32 = mybir.dt.float32
BF16 = mybir.dt.bfloat16
FP8E3 = mybir.dt.float8e3
U32 = mybir.dt.uint32
GELU = mybir.ActivationFunctionType.Gelu
EXP = mybir.ActivationFunctionType.Exp
X_AX = mybir.AxisListType.X
ADD = mybir.AluOpType.add
bf16 = ml_dtypes.bfloat16
f8e3 = ml_dtypes.float8_e3m4

W2DT = FP8E3 if W2_FP8 else BF16


def _build(include_bias: bool) -> bass.Bass:
    nc = bacc.Bacc("TRN2", num_devices=NCORES)

    # weight stream (sync ring), packed layouts produced by _pack_inputs
    swd = nc.dram_tensor("swd", [NSW * P, SWPACK * SWC], BF16, kind="ExternalInput")
    w1d = nc.dram_tensor("w1d", [NW1 * P, W1PACK * KD], BF16, kind="ExternalInput")
    w2d = nc.dram_tensor("w2d", [NJ * W2SUB * P, W2K * JW], W2DT, kind="ExternalInput")
    # everything else (scalar/gpsimd rings)
    xt = nc.dram_tensor("xt", [P, 3, NTOK + 1], F32, kind="ExternalInput")
    x2b = nc.dram_tensor("x2b", [NTOK, D], BF16, kind="ExternalInput")
    xft = nc.dram_tensor("xft", [P, KCS, B], BF16, kind="ExternalInput")
    sw2 = nc.dram_tensor("sw2", [P, KCH, E], BF16, kind="ExternalInput")
    ch1 = nc.dram_tensor("ch1", [P, KCE * CH1C], BF16, kind="ExternalInput")
    ch2 = nc.dram_tensor("ch2", [P, 3, C], BF16, kind="ExternalInput")
    oh = nc.dram_tensor("oh", [B, E], F32, kind="ExternalInput")
    if include_bias:
        b1d = nc.dram_tensor("b1d", [1, KD], F32, kind="ExternalInput")
        b2d = nc.dram_tensor("b2d", [1, KD], F32, kind="ExternalInput")  # pre-scaled
        swb1d = nc.dram_tensor("swb1d", [1, SWC], F32, kind="ExternalInput")
        swb2d = nc.dram_tensor("swb2d", [1, E], F32, kind="ExternalInput")
        chb1d = nc.dram_tensor("chb1d", [1, CH1C], F32, kind="ExternalInput")
    outp = nc.dram_tensor("outp", [B, C], F32, kind="ExternalOutput")

    with tile.TileContext(nc) as tc:
        with (
            tc.tile_pool(name="consts", bufs=1) as consts,
            tc.tile_pool(name="acts", bufs=1) as acts,
            tc.tile_pool(name="wpool", bufs=7) as wpool,
            tc.tile_pool(name="ps", bufs=6, space="PSUM") as ps,
            tc.tile_pool(name="dram", bufs=1, space="DRAM") as dram,
        ):
            # ---- constants / inputs on the scalar+gpsimd rings ----
            ident = consts.tile([P, P], BF16)
            make_identity(nc, ident[:])
            xt_sb = acts.tile([P, 3, NTOK + 1], F32)
            nc.scalar.dma_start(xt_sb[:], xt[:])
            xft_sb = consts.tile([P, KCS, B], BF16)
            nc.scalar.dma_start(xft_sb[:], xft[:])
            sw2_sb = consts.tile([P, KCH, E], BF16)
            nc.scalar.dma_start(sw2_sb[:], sw2[:])
            oh_sb = consts.tile([B, E], F32)
            nc.scalar.dma_start(oh_sb[:], oh[:])
            ch2_sb = consts.tile([P, 3, C], BF16)
            nc.scalar.dma_start(ch2_sb[:], ch2[:])
            if include_bias:
                b1_sb = consts.tile([B, KD], F32)
                nc.scalar.dma_start(b1_sb[:], b1d[0:1, :].to_broadcast([B, KD]))
                b2_sb = consts.tile([B, KD], F32)
                nc.scalar.dma_start(b2_sb[:], b2d[0:1, :].to_broadcast([B, KD]))
                swb1_sb = consts.tile([B, SWC], F32)
                nc.scalar.dma_start(swb1_sb[:], swb1d[0:1, :].to_broadcast([B, SWC]))
                swb2_sb = consts.tile([B, E], F32)
                nc.scalar.dma_start(swb2_sb[:], swb2d[0:1, :].to_broadcast([B, E]))
                chb1_sb = consts.tile([B, CH1C], F32)
                nc.scalar.dma_start(chb1_sb[:], chb1d[0:1, :].to_broadcast([B, CH1C]))

            pwarm = ps.tile([P, B], BF16, name="pwarm", tag="pt", bufs=2)
            nc.tensor.transpose(pwarm[:32, :32], ident[:32, :32], ident[:32, :32])

            # ---- sum-weights GEMM1: stream sw1, accumulate 3 banks ----
            pms = [ps.tile([B, 512], F32, name=f"pms{n}", tag="pm")
                   for n in range(3)]
            for t in range(NSW):
                wt = wpool.tile([P, SWPACK * SWC], BF16, name="wt", tag="wt")
                nc.sync.dma_start(wt[:], swd[ts(t, P), :])
                for k in range(SWPACK):
                    c = t * SWPACK + k
                    for n in range(3):
                        nc.tensor.matmul(
                            pms[n][:], xft_sb[:, c, :],
                            wt[:, ds(k * SWC + n * 512, 512)],
                            start=(c == 0), stop=(c == KCS - 1),
                        )
                if t == 1:
                    # ---- router (after xt lands): logits = x @ emb_e ----
                    lg_flat = acts.tile([1, NTOK], F32)
                    for nt in range(4):
                        pr = ps.tile([B, 512], F32, name="pr", tag="pt", bufs=2)
                        for cc in range(3):
                            nc.tensor.matmul(
                                pr[:1, :], xt_sb[:, cc, NTOK : NTOK + 1],
                                xt_sb[:, cc, ts(nt, 512)],
                                start=(cc == 0), stop=(cc == 2),
                            )
                        nc.vector.tensor_copy(lg_flat[:, ts(nt, 512)], pr[:1, :])
                    lg_dram = dram.tile([1, NTOK], F32)
                    nc.scalar.dma_start(lg_dram[:], lg_flat[:])
                    lg_bn = acts.tile([B, N], F32)
                    nc.scalar.dma_start(
                        lg_bn[:], lg_dram[:].rearrange("x (b n) -> (x b) n", b=B))
                if t == 2:
                    # ---- top-8 per row + token gather ----
                    vals8 = acts.tile([B, 8], F32)
                    idx8 = acts.tile([B, 8], U32)
                    nc.vector.max(out=vals8[:], in_=lg_bn[:])
                    nc.vector.max_index(out=idx8[:], in_max=vals8[:], in_values=lg_bn[:])
                    base = acts.tile([B, 1], U32)
                    nc.gpsimd.iota(base[:], pattern=[[0, 1]], base=0, channel_multiplier=N)
                    off = acts.tile([B, 8], U32)
                    nc.vector.tensor_tensor(
                        out=off[:], in0=idx8[:], in1=base[:].to_broadcast([B, 8]), op=ADD)
                    sel = acts.tile([B, K, D], BF16)
                    for k in range(K):
                        nc.gpsimd.indirect_dma_start(
                            out=sel[:, k, :], out_offset=None,
                            in_=x2b[:],
                            in_offset=bass.IndirectOffsetOnAxis(ap=off[:, k : k + 1], axis=0),
                        )
                if t == 3:
                    # selT chunks [128, 24, 64] for expert GEMM1 stationaries
                    sel_flat = sel[:].rearrange("b k d -> b (k d)")
                    selT = acts.tile([P, KCE, B], BF16)
                    for c in range(KCE):
                        pt = ps.tile([P, B], BF16, name="pt", tag="pt", bufs=2)
                        nc.tensor.transpose(pt[:], sel_flat[:, ts(c, P)], ident[:B, :B])
                        nc.vector.tensor_copy(selT[:, c, :], pt[:])

            # ---- z = h1 @ sw2 partials, AllReduce (tiny, fp32) ----
            h1 = acts.tile([B, SWC], BF16)
            for n in range(3):
                if include_bias:
                    nc.vector.tensor_add(pms[n][:], pms[n][:], swb1_sb[:, ts(n, 512)])
                nc.scalar.activation(h1[:, ts(n, 512)], pms[n][:], GELU)
            h1T = acts.tile([P, KCH, B], BF16)
            for c in range(KCH):
                pt = ps.tile([P, B], BF16, name="pt", tag="pt", bufs=2)
                nc.tensor.transpose(pt[:], h1[:, ts(c, P)], ident[:B, :B])
                nc.vector.tensor_copy(h1T[:, c, :], pt[:])
            pz = ps.tile([B, 512], F32, name="pz", tag="pm")
            for c in range(KCH):
                nc.tensor.matmul(
                    pz[:E, :B], sw2_sb[:, c, :], h1T[:, c, :],
                    start=(c == 0), stop=(c == KCH - 1),
                )
            zT_sb = acts.tile([E, B], F32)
            nc.vector.tensor_copy(zT_sb[:], pz[:E, :B])
            zin = dram.tile([E, B], F32)
            zout = dram.tile([E, B], F32)
            nc.scalar.dma_start(zin[:], zT_sb[:])
            nc.gpsimd.collective_compute(
                "AllReduce", ADD, replica_groups=[list(range(NCORES))],
                ins=[zin[:].opt()], outs=[zout[:].opt()],
            )

            # ---- expert GEMM1: h = gelu(selT.T @ w1_e) ----
            h = acts.tile([B, KD], BF16)
            pme = [ps.tile([B, 512], F32, name=f"pme{n}", tag="pm")
                   for n in range(6)]
            for t in range(NW1):
                wt = wpool.tile([P, W1PACK * KD], BF16, name="wt", tag="wt")
                nc.sync.dma_start(wt[:], w1d[ts(t, P), :])
                if t == 1:
                    # prefetch ch1 on the scalar ring (needed from the head on)
                    ch1_sb = consts.tile([P, KCE * CH1C], BF16)
                    nc.scalar.dma_start(ch1_sb[:], ch1[:])
                for k in range(W1PACK):
                    c = t * W1PACK + k
                    for n in range(6):
                        nc.tensor.matmul(
                            pme[n][:], selT[:, c, :],
                            wt[:, ds(k * KD + n * 512, 512)],
                            start=(c == 0), stop=(c == KCE - 1),
                        )
            for n in range(6):
                if include_bias:
                    nc.vector.tensor_add(pme[n][:], pme[n][:], b1_sb[:, ts(n, 512)])
                nc.scalar.activation(h[:, ts(n, 512)], pme[n][:], GELU)
            hT = acts.tile([P, KCE, B], BF16)
            for c in range(KCE):
                pt = ps.tile([P, B], BF16, name="pt", tag="pt", bufs=2)
                nc.tensor.transpose(pt[:], h[:, ts(c, P)], ident[:B, :B])
                nc.vector.tensor_copy(hT[:, c, :], pt[:])

            # ---- softmax over experts; wes = w[:, e] / W2_SCALE.  Emitted
            # after the h gelus/hT copies so no z-dependent op can park the
            # ACT or DVE FIFOs while GEMM1 output processing is pending. ----
            zb = acts.tile([B, E], F32)
            nc.gpsimd.dma_start(zb[:], zout[:].rearrange("e b -> b e"))
            if include_bias:
                nc.vector.tensor_add(zb[:], zb[:], swb2_sb[:])
            mx = acts.tile([B, 1], F32)
            nc.vector.reduce_max(mx[:], zb[:], axis=X_AX)
            nmx = acts.tile([B, 1], F32)
            nc.vector.tensor_scalar_mul(nmx[:], mx[:], -1.0)
            exps = acts.tile([B, E], F32)
            nc.scalar.activation(exps[:], zb[:], EXP, bias=nmx[:])
            sm = acts.tile([B, 1], F32)
            nc.vector.reduce_sum(sm[:], exps[:], axis=X_AX)
            rs = acts.tile([B, 1], F32)
            nc.vector.reciprocal(rs[:], sm[:])
            wv = acts.tile([B, E], F32)
            nc.vector.tensor_scalar_mul(wv[:], exps[:], rs[:])
            t8 = acts.tile([B, E], F32)
            nc.vector.tensor_mul(out=t8[:], in0=wv[:], in1=oh_sb[:])
            wes = acts.tile([B, 1], F32)
            nc.vector.reduce_sum(wes[:], t8[:], axis=X_AX)
            if W2_FP8:
                nc.vector.tensor_scalar_mul(wes[:], wes[:], 1.0 / W2_SCALE)

            # ---- expert GEMM2 in NJ column chunks (PSUM reuse); ONE bf16
            # AllReduce for the whole [B, KD] er (chunked ARs would only
            # serialize on the CC engine behind the z-AR anyway, and each
            # extra AR pays its own mesh latency). ----
            cin = dram.tile([B, KD], BF16, name="cin")
            wsout = dram.tile([B, KD], BF16, name="wsout")
            er_bf = acts.tile([B, KD], BF16)
            for j in range(NJ):
                pme2 = [ps.tile([B, 512], F32, name=f"pme2{j}{n}", tag="pm")
                        for n in range(2)]
                for s in range(W2SUB):
                    wt = wpool.tile([P, W2K * JW], W2DT, name="wt", tag="wt")
                    nc.sync.dma_start(wt[:], w2d[ts(j * W2SUB + s, P), :])
                    for k in range(W2K):
                        c = s * W2K + k
                        for n in range(2):
                            nc.tensor.matmul(
                                pme2[n][:], hT[:, c, :],
                                wt[:, ds(k * JW + n * 512, 512)],
                                start=(c == 0), stop=(c == KCE - 1),
                            )
                for n in range(2):
                    if include_bias:
                        nc.vector.tensor_add(
                            pme2[n][:], pme2[n][:], b2_sb[:, ds(j * JW + n * 512, 512)])
                    nc.vector.tensor_scalar_mul(
                        er_bf[:, ds(j * JW + n * 512, 512)], pme2[n][:], wes[:])
                nc.scalar.dma_start(cin[:, ds(j * JW, JW)], er_bf[:, ds(j * JW, JW)])
                if j == 0:
                    pmh = ps.tile([B, 512], F32, name="pmh", tag="pm")
            nc.gpsimd.collective_compute(
                "AllReduce", ADD, replica_groups=[list(range(NCORES))],
                ins=[cin[:].opt()], outs=[wsout[:].opt()],
            )

            # ---- head GEMM1 (column shard) ----
            ws_sb = acts.tile([B, KD], BF16)
            wsT = acts.tile([P, KCE, B], BF16)
            nc.scalar.dma_start(ws_sb[:], wsout[:])
            for c in range(KCE):
                pt = ps.tile([P, B], BF16, name="pt", tag="pt", bufs=2)
                nc.tensor.transpose(pt[:], ws_sb[:, ts(c, P)], ident[:B, :B])
                nc.vector.tensor_copy(wsT[:, c, :], pt[:])
                nc.tensor.matmul(
                    pmh[:, :CH1C], wsT[:, c, :],
                    ch1_sb[:, ds(c * CH1C, CH1C)],
                    start=(c == 0), stop=(c == KCE - 1),
                )
            hh = acts.tile([B, CH1C], BF16)
            if include_bias:
                nc.vector.tensor_add(pmh[:, :CH1C], pmh[:, :CH1C], chb1_sb[:])
            nc.scalar.activation(hh[:], pmh[:, :CH1C], GELU)

            # ---- head GEMM2 (contraction shard): out_part = hh @ ch2_e ----
            hhT = acts.tile([P, 3, B], BF16)
            for c in range(3):
                pt = ps.tile([P, B], BF16, name="pt", tag="pt", bufs=2)
                nc.tensor.transpose(pt[:], hh[:, ts(c, P)], ident[:B, :B])
                nc.vector.tensor_copy(hhT[:, c, :], pt[:])
            outsb = acts.tile([B, C], F32)
            for nn in range(2):
                pmo = ps.tile([B, 512], F32, name="pmo", tag="pm")
                for c in range(3):
                    nc.tensor.matmul(
                        pmo[:, :500], hhT[:, c, :], ch2_sb[:, c, ds(nn * 500, 500)],
                        start=(c == 0), stop=(c == 2),
                    )
                nc.vector.tensor_copy(outsb[:, ds(nn * 500, 500)], pmo[:, :500])
            nc.scalar.dma_start(outp[:], outsb[:])

    nc.finalize()
    return nc


_NC_CACHE: dict = {}


def _get_nc(include_bias: bool) -> bass.Bass:
    if include_bias not in _NC_CACHE:
        _NC_CACHE[include_bias] = _build(include_bias)
    return _NC_CACHE[include_bias]


def _pack_w2(w2_e: np.ndarray) -> np.ndarray:
    """[3072, 3072] -> [NJ*W2SUB*128, W2K*1024] in (j, sub, k-interleaved) layout."""
    cols = w2_e.reshape(KD, NJ, JW)                       # split columns
    out = np.empty((NJ * W2SUB * P, W2K * JW), dtype=np.float32)
    for j in range(NJ):
        blk = cols[:, j, :].reshape(W2SUB, W2K, P, JW)    # [sub, k, p, jw]
        out[j * W2SUB * P:(j + 1) * W2SUB * P] = (
            blk.transpose(0, 2, 1, 3).reshape(W2SUB * P, W2K * JW))
    if W2_FP8:
        m = float(ml_dtypes.finfo(f8e3).max)
        return np.clip(out * W2_SCALE, -m, m).astype(f8e3)
    return out.astype(bf16)


def _pack_inputs(inputs: dict, include_bias: bool) -> list[dict]:
    f32 = np.float32
    x = np.ascontiguousarray(np.asarray(inputs["x"], dtype=f32))      # (64,32,384)
    expert_emb = np.asarray(inputs["expert_emb"], dtype=f32)          # (8,384)
    w1 = np.asarray(inputs["w1"])                                     # (8,3072,3072)
    w2 = np.asarray(inputs["w2"])
    sw_w1 = np.asarray(inputs["sw_w1"])                               # (12288,12288)
    sw_w2 = np.asarray(inputs["sw_w2"])                               # (12288,8)
    ch_w1 = np.asarray(inputs["ch_w1"])                               # (3072,3072)
    ch_w2 = np.asarray(inputs["ch_w2"])                               # (3072,1000)

    x2 = x.reshape(NTOK, D)
    xt_base = x2.T.reshape(3, P, NTOK).transpose(1, 0, 2)             # (128,3,2048)
    x2b = x2.astype(bf16)                                             # (2048,384)
    xf = x.reshape(B, ND)
    xft_p = np.ascontiguousarray(
        xf.T.reshape(KCS, P, B).transpose(1, 0, 2)).astype(bf16)      # (128,96,64)

    ch1_full = ch_w1.reshape(KD, E, CH1C)                             # col shards
    ch2_full = ch_w2.reshape(E, CH1C, C)                              # row shards

    in_maps = []
    for e in range(NCORES):
        emb_p = expert_emb[e].reshape(3, P).T                          # (128,3)
        xt_p = np.ascontiguousarray(
            np.concatenate([xt_base, emb_p[:, :, None]], axis=2), dtype=f32)
        sw1_e = sw_w1[:, e * SWC:(e + 1) * SWC]                        # (12288,1536)
        swd_p = np.ascontiguousarray(
            sw1_e.reshape(NSW, SWPACK, P, SWC).transpose(0, 2, 1, 3)
            .reshape(NSW * P, SWPACK * SWC)).astype(bf16)
        w1d_p = np.ascontiguousarray(
            np.asarray(w1[e], f32).reshape(NW1, W1PACK, P, KD)
            .transpose(0, 2, 1, 3).reshape(NW1 * P, W1PACK * KD)).astype(bf16)
        w2d_p = _pack_w2(np.asarray(w2[e], f32))
        sw2_e = np.ascontiguousarray(sw_w2[e * SWC:(e + 1) * SWC, :])  # (1536,8)
        sw2_p = np.ascontiguousarray(
            sw2_e.reshape(KCH, P, E).transpose(1, 0, 2)).astype(bf16)  # (128,12,8)
        ch1_p = np.ascontiguousarray(
            ch1_full[:, e, :].reshape(KCE, P, CH1C).transpose(1, 0, 2)
            .reshape(P, KCE * CH1C)).astype(bf16)                      # (128,24*384)
        ch2_p = np.ascontiguousarray(
            ch2_full[e].reshape(3, P, C).transpose(1, 0, 2)).astype(bf16)  # (128,3,1000)
        oh_p = np.zeros((B, E), dtype=f32)
        oh_p[:, e] = 1.0
        m = {
            "xt": xt_p, "x2b": x2b, "xft": xft_p,
            "swd": swd_p, "w1d": w1d_p, "w2d": w2d_p,
            "sw2": sw2_p, "ch1": ch1_p, "ch2": ch2_p, "oh": oh_p,
        }
        if include_bias:
            m["b1d"] = np.asarray(inputs["b1"][e], f32).reshape(1, KD)
            b2v = np.asarray(inputs["b2"][e], f32).reshape(1, KD)
            m["b2d"] = b2v * (W2_SCALE if W2_FP8 else 1.0)
            m["swb1d"] = np.asarray(
                inputs["sw_b1"], f32).reshape(1, ND)[:, e * SWC:(e + 1) * SWC]
            m["swb2d"] = np.asarray(inputs["sw_b2"], f32).reshape(1, E)
            m["chb1d"] = np.asarray(
                inputs["ch_b1"], f32).reshape(1, KD)[:, e * CH1C:(e + 1) * CH1C]
        in_maps.append(m)
    return in_maps


def _need_bias(inputs) -> bool:
    return any(
        float(np.abs(np.asarray(inputs[k])).max()) != 0.0
        for k in ("b1", "b2", "sw_b1", "sw_b2", "ch_b1")
    )


def run(inputs: dict, **run_kwargs):
    """Run on the 8 cores; returns (full_output, BassKernelResults)."""
    include_bias = _need_bias(inputs)
    nc = _get_nc(include_bias)
    in_maps = _pack_inputs(inputs, include_bias)
    res = run_bass_kernel_spmd(nc, in_maps, core_ids=list(range(NCORES)), **run_kwargs)
    out = np.zeros((B, C), dtype=np.float64)
    for e in range(NCORES):
        out += res.results[e]["outp"].astype(np.float64)
    out += np.asarray(inputs["ch_b2"], np.float64)
    return out.astype(np.float32), res


def kernel(**inputs) -> np.ndarray:
    out, _ = run(inputs)
    return out


# revision 8
# speedup vs baseline: 1.0110x; 1.0110x over previous
"""Expert-choice MoE kernel for 8 Trainium2 NeuronCores (expert-parallel).

Decomposition (core e handles expert e):
  - router logits x . emb_e computed in fp32 on PE; top-8 token indices per
    batch row via DVE max8/max_index; token gather via indirect DMA.
  - sum_weights GEMM1 column-sharded (each core owns 1536 columns of sw_w1);
    the tiny (8,64) partial logits are AllReduced, softmaxed locally.
  - expert MLP: GEMM1 (w1) in bf16; GEMM2 (w2) weights streamed as fp8e3m4
    scaled x128 (descale folded into the combine weights `wes`).
  - er * w[:, e] contributions AllReduced in bf16, in 3 column chunks of
    1024 that pipeline with the w2 stream; each arrived ws chunk feeds the
    (column-sharded) head GEMM1 accumulation immediately.
  - classification head sharded: GEMM1 column-shard (384 cols of ch_w1),
    GEMM2 contraction-shard (384 rows of ch_w2); per-core (64,1000) partials
    are summed on the host (+ ch_b2).

DMA ring assignment: sync(HWDGE/SP) ring carries ONLY the big weight
stream (sw1 -> w1 -> w2) in consumption order, packed into large
contiguous chunks (1.5MB sw / 1.5MB w1 / 1MB w2). Activations, inputs,
ch1/ch2 and collective staging ride the scalar(Act) ring; gathers and
collective triggers ride gpsimd (SWDGE).

PSUM plan (8 banks): tag "pm" bufs=6 + tag "pt" bufs=2 (transposes +
router). The "pm" ring allocation order is load-bearing (ring reuse must
only ever land on a dead tile): pms x3, pz, pme x6, pme2_j0 x2, pmh,
pme2_j1 x2, pme2_j2 x2, pmo x2 (single er-AR after all GEMM2 chunks).
"""

import numpy as np
import ml_dtypes

import concourse.bass as bass
from concourse import bacc
import concourse.mybir as mybir
import concourse.tile as tile
from concourse.bass import ts, ds
from concourse.bass_utils import run_bass_kernel_spmd
from concourse.masks import make_identity

B, N, D, E, K, C = 64, 32, 384, 8, 8, 1000
KD, ND = K * D, N * D          # 3072, 12288
P = 128
NTOK = B * N                   # 2048
SWC = ND // E                  # 1536 sum-weights columns per core
CH1C = KD // E                 # 384 head-GEMM1 columns per core
KCE = KD // P                  # 24 k-chunks, expert GEMMs
KCS = ND // P                  # 96 k-chunks, sum-weights GEMM1
KCH = SWC // P                 # 12 k-chunks, z GEMM
NCORES = 8

SWPACK = 4                     # k-chunks per sw1 DMA (1.5MB)
NSW = KCS // SWPACK            # 24 sw tiles
W1PACK = 2                     # k-chunks per w1 DMA (1.5MB)
NW1 = KCE // W1PACK            # 12 w1 tiles
NJ = 3                         # er/AllReduce column chunks
JW = KD // NJ                  # 1024 columns per chunk
W2SUB = 3                      # sub-DMAs per w2 column chunk
W2K = KCE // W2SUB             # 8 k-chunks per w2 sub-DMA

W2_FP8 = True                  # stream w2 as fp8e3m4 (scale 128)
W2_SCALE = 128.0

F32 = mybir.dt.float32
BF16 = mybir.dt.bfloat16
FP8E3 = mybir.dt.float8e3
U32 = mybir.dt.uint32
GELU = mybir.ActivationFunctionType.Gelu
EXP = mybir.ActivationFunctionType.Exp
X_AX = mybir.AxisListType.X
ADD = mybir.AluOpType.add
bf16 = ml_dtypes.bfloat16
f8e3 = ml_dtypes.float8_e3m4

W2DT = FP8E3 if W2_FP8 else BF16


def _build(include_bias: bool) -> bass.Bass:
    nc = bacc.Bacc("TRN2", num_devices=NCORES)

    # weight stream (sync ring), packed layouts produced by _pack_inputs
    swd = nc.dram_tensor("swd", [NSW * P, SWPACK * SWC], BF16, kind="ExternalInput")
    w1d = nc.dram_tensor("w1d", [NW1 * P, W1PACK * KD], BF16, kind="ExternalInput")
    w2d = nc.dram_tensor("w2d", [NJ * W2SUB * P, W2K * JW], W2DT, kind="ExternalInput")
    # everything else (scalar/gpsimd rings)
    xt = nc.dram_tensor("xt", [P, 3, NTOK + 1], F32, kind="ExternalInput")
    x2b = nc.dram_tensor("x2b", [NTOK, D], BF16, kind="ExternalInput")
    xft = nc.dram_tensor("xft", [P, KCS, B], BF16, kind="ExternalInput")
    sw2 = nc.dram_tensor("sw2", [P, KCH, E], BF16, kind="ExternalInput")
    ch1 = nc.dram_tensor("ch1", [P, KCE * CH1C], BF16, kind="ExternalInput")
    ch2 = nc.dram_tensor("ch2", [P, 3, C], BF16, kind="ExternalInput")
    oh = nc.dram_tensor("oh", [B, E], F32, kind="ExternalInput")
    if include_bias:
        b1d = nc.dram_tensor("b1d", [1, KD], F32, kind="ExternalInput")
        b2d = nc.dram_tensor("b2d", [1, KD], F32, kind="ExternalInput")  # pre-scaled
        swb1d = nc.dram_tensor("swb1d", [1, SWC], F32, kind="ExternalInput")
        swb2d = nc.dram_tensor("swb2d", [1, E], F32, kind="ExternalInput")
        chb1d = nc.dram_tensor("chb1d", [1, CH1C], F32, kind="ExternalInput")
    outp = nc.dram_tensor("outp", [B, C], F32, kind="ExternalOutput")

    with tile.TileContext(nc) as tc:
        with (
            tc.tile_pool(name="consts", bufs=1) as consts,
            tc.tile_pool(name="acts", bufs=1) as acts,
            tc.tile_pool(name="wpool", bufs=7) as wpool,
            tc.tile_pool(name="ps", bufs=6, space="PSUM") as ps,
            tc.tile_pool(name="dram", bufs=1, space="DRAM") as dram,
        ):
            # ---- constants / inputs on the scalar+gpsimd rings ----
            ident = consts.tile([P, P], BF16)
            make_identity(nc, ident[:])
            xft_sb = consts.tile([P, KCS, B], BF16)
            nc.scalar.dma_start(xft_sb[:], xft[:])
            xt_sb = acts.tile([P, 3, NTOK + 1], F32)
            nc.scalar.dma_start(xt_sb[:], xt[:])
            sw2_sb = consts.tile([P, KCH, E], BF16)
            nc.scalar.dma_start(sw2_sb[:], sw2[:])
            oh_sb = consts.tile([B, E], F32)
            nc.scalar.dma_start(oh_sb[:], oh[:])
            ch2_sb = consts.tile([P, 3, C], BF16)
            nc.scalar.dma_start(ch2_sb[:], ch2[:])
            if include_bias:
                b1_sb = consts.tile([B, KD], F32)
                nc.scalar.dma_start(b1_sb[:], b1d[0:1, :].to_broadcast([B, KD]))
                b2_sb = consts.tile([B, KD], F32)
                nc.scalar.dma_start(b2_sb[:], b2d[0:1, :].to_broadcast([B, KD]))
                swb1_sb = consts.tile([B, SWC], F32)
                nc.scalar.dma_start(swb1_sb[:], swb1d[0:1, :].to_broadcast([B, SWC]))
                swb2_sb = consts.tile([B, E], F32)
                nc.scalar.dma_start(swb2_sb[:], swb2d[0:1, :].to_broadcast([B, E]))
                chb1_sb = consts.tile([B, CH1C], F32)
                nc.scalar.dma_start(chb1_sb[:], chb1d[0:1, :].to_broadcast([B, CH1C]))

            pwarm = ps.tile([P, B], BF16, name="pwarm", tag="pt", bufs=2)
            nc.tensor.transpose(pwarm[:32, :32], ident[:32, :32], ident[:32, :32])

            # ---- sum-weights GEMM1: stream sw1, accumulate 3 banks ----
            pms = [ps.tile([B, 512], F32, name=f"pms{n}", tag="pm")
                   for n in range(3)]
            for t in range(NSW):
                wt = wpool.tile([P, SWPACK * SWC], BF16, name="wt", tag="wt")
                nc.sync.dma_start(wt[:], swd[ts(t, P), :])
                for k in range(SWPACK):
                    c = t * SWPACK + k
                    for n in range(3):
                        nc.tensor.matmul(
                            pms[n][:], xft_sb[:, c, :],
                            wt[:, ds(k * SWC + n * 512, 512)],
                            start=(c == 0), stop=(c == KCS - 1),
                        )
                if t == 1:
                    # ---- router (after xt lands): logits = x @ emb_e ----
                    lg_flat = acts.tile([1, NTOK], F32)
                    for nt in range(4):
                        pr = ps.tile([B, 512], F32, name="pr", tag="pt", bufs=2)
                        for cc in range(3):
                            nc.tensor.matmul(
                                pr[:1, :], xt_sb[:, cc, NTOK : NTOK + 1],
                                xt_sb[:, cc, ts(nt, 512)],
                                start=(cc == 0), stop=(cc == 2),
                            )
                        nc.vector.tensor_copy(lg_flat[:, ts(nt, 512)], pr[:1, :])
                    lg_dram = dram.tile([1, NTOK], F32)
                    nc.scalar.dma_start(lg_dram[:], lg_flat[:])
                    lg_bn = acts.tile([B, N], F32)
                    nc.scalar.dma_start(
                        lg_bn[:], lg_dram[:].rearrange("x (b n) -> (x b) n", b=B))
                if t == 2:
                    # ---- top-8 per row + token gather ----
                    vals8 = acts.tile([B, 8], F32)
                    idx8 = acts.tile([B, 8], U32)
                    nc.vector.max(out=vals8[:], in_=lg_bn[:])
                    nc.vector.max_index(out=idx8[:], in_max=vals8[:], in_values=lg_bn[:])
                    base = acts.tile([B, 1], U32)
                    nc.gpsimd.iota(base[:], pattern=[[0, 1]], base=0, channel_multiplier=N)
                    off = acts.tile([B, 8], U32)
                    nc.vector.tensor_tensor(
                        out=off[:], in0=idx8[:], in1=base[:].to_broadcast([B, 8]), op=ADD)
                    sel = acts.tile([B, K, D], BF16)
                    for k in range(K):
                        nc.gpsimd.indirect_dma_start(
                            out=sel[:, k, :], out_offset=None,
                            in_=x2b[:],
                            in_offset=bass.IndirectOffsetOnAxis(ap=off[:, k : k + 1], axis=0),
                        )
                if t == 3:
                    # selT chunks [128, 24, 64] for expert GEMM1 stationaries
                    sel_flat = sel[:].rearrange("b k d -> b (k d)")
                    selT = acts.tile([P, KCE, B], BF16)
                    for c in range(KCE):
                        pt = ps.tile([P, B], BF16, name="pt", tag="pt", bufs=2)
                        nc.tensor.transpose(pt[:], sel_flat[:, ts(c, P)], ident[:B, :B])
                        nc.vector.tensor_copy(selT[:, c, :], pt[:])

            # ---- z = h1 @ sw2 partials, AllReduce (tiny, fp32) ----
            h1 = acts.tile([B, SWC], BF16)
            for n in range(3):
                if include_bias:
                    nc.vector.tensor_add(pms[n][:], pms[n][:], swb1_sb[:, ts(n, 512)])
                nc.scalar.activation(h1[:, ts(n, 512)], pms[n][:], GELU)
            h1T = acts.tile([P, KCH, B], BF16)
            for c in range(KCH):
                pt = ps.tile([P, B], BF16, name="pt", tag="pt", bufs=2)
                nc.tensor.transpose(pt[:], h1[:, ts(c, P)], ident[:B, :B])
                nc.vector.tensor_copy(h1T[:, c, :], pt[:])
            pz = ps.tile([B, 512], F32, name="pz", tag="pm")
            for c in range(KCH):
                nc.tensor.matmul(
                    pz[:E, :B], sw2_sb[:, c, :], h1T[:, c, :],
                    start=(c == 0), stop=(c == KCH - 1),
                )
            zT_sb = acts.tile([E, B], F32)
            nc.vector.tensor_copy(zT_sb[:], pz[:E, :B])
            zin = dram.tile([E, B], F32)
            zout = dram.tile([E, B], F32)
            nc.scalar.dma_start(zin[:], zT_sb[:])
            nc.gpsimd.collective_compute(
                "AllReduce", ADD, replica_groups=[list(range(NCORES))],
                ins=[zin[:].opt()], outs=[zout[:].opt()],
            )

            # ---- expert GEMM1: h = gelu(selT.T @ w1_e) ----
            h = acts.tile([B, KD], BF16)
            pme = [ps.tile([B, 512], F32, name=f"pme{n}", tag="pm")
                   for n in range(6)]
            for t in range(NW1):
                wt = wpool.tile([P, W1PACK * KD], BF16, name="wt", tag="wt")
                nc.sync.dma_start(wt[:], w1d[ts(t, P), :])
                if t == 1:
                    # prefetch ch1 on the scalar ring (needed from the head on)
                    ch1_sb = consts.tile([P, KCE * CH1C], BF16)
                    nc.scalar.dma_start(ch1_sb[:], ch1[:])
                for k in range(W1PACK):
                    c = t * W1PACK + k
                    for n in range(6):
                        nc.tensor.matmul(
                            pme[n][:], selT[:, c, :],
                            wt[:, ds(k * KD + n * 512, 512)],
                            start=(c == 0), stop=(c == KCE - 1),
                        )
            last_gelu = None
            for n in range(6):
                if include_bias:
                    nc.vector.tensor_add(pme[n][:], pme[n][:], b1_sb[:, ts(n, 512)])
                last_gelu = nc.scalar.activation(h[:, ts(n, 512)], pme[n][:], GELU)
            hT = acts.tile([P, KCE, B], BF16)
            last_htc = None
            for c in range(KCE):
                pt = ps.tile([P, B], BF16, name="pt", tag="pt", bufs=2)
                nc.tensor.transpose(pt[:], h[:, ts(c, P)], ident[:B, :B])
                last_htc = nc.vector.tensor_copy(hT[:, c, :], pt[:])

            # ---- softmax over experts; wes = w[:, e] / W2_SCALE.  Emitted
            # after the h gelus/hT copies so no z-dependent op can park the
            # ACT or DVE FIFOs while GEMM1 output processing is pending. ----
            zb = acts.tile([B, E], F32)
            nc.gpsimd.dma_start(zb[:], zout[:].rearrange("e b -> b e"))
            if include_bias:
                nc.vector.tensor_add(zb[:], zb[:], swb2_sb[:])
            mx = acts.tile([B, 1], F32)
            mx_i = nc.vector.reduce_max(mx[:], zb[:], axis=X_AX)
            tile.add_dep_helper(mx_i.ins, last_htc.ins, sync=False,
                                reason="softmax after hT copies on DVE")
            nmx = acts.tile([B, 1], F32)
            nc.vector.tensor_scalar_mul(nmx[:], mx[:], -1.0)
            exps = acts.tile([B, E], F32)
            exp_i = nc.scalar.activation(exps[:], zb[:], EXP, bias=nmx[:])
            tile.add_dep_helper(exp_i.ins, last_gelu.ins, sync=False,
                                reason="Exp after expert gelus on ACT")
            sm = acts.tile([B, 1], F32)
            nc.vector.reduce_sum(sm[:], exps[:], axis=X_AX)
            rs = acts.tile([B, 1], F32)
            nc.vector.reciprocal(rs[:], sm[:])
            wv = acts.tile([B, E], F32)
            nc.vector.tensor_scalar_mul(wv[:], exps[:], rs[:])
            t8 = acts.tile([B, E], F32)
            nc.vector.tensor_mul(out=t8[:], in0=wv[:], in1=oh_sb[:])
            wes = acts.tile([B, 1], F32)
            nc.vector.reduce_sum(wes[:], t8[:], axis=X_AX)
            if W2_FP8:
                nc.vector.tensor_scalar_mul(wes[:], wes[:], 1.0 / W2_SCALE)

            # ---- expert GEMM2 in NJ column chunks (PSUM reuse); ONE bf16
            # AllReduce for the whole [B, KD] er (chunked ARs would only
            # serialize on the CC engine behind the z-AR anyway, and each
            # extra AR pays its own mesh latency). ----
            cin = dram.tile([B, KD], BF16, name="cin")
            wsout = dram.tile([B, KD], BF16, name="wsout")
            er_bf = acts.tile([B, KD], BF16)
            for j in range(NJ):
                pme2 = [ps.tile([B, 512], F32, name=f"pme2{j}{n}", tag="pm")
                        for n in range(2)]
                for s in range(W2SUB):
                    wt = wpool.tile([P, W2K * JW], W2DT, name="wt", tag="wt")
                    nc.sync.dma_start(wt[:], w2d[ts(j * W2SUB + s, P), :])
                    for k in range(W2K):
                        c = s * W2K + k
                        for n in range(2):
                            nc.tensor.matmul(
                                pme2[n][:], hT[:, c, :],
                                wt[:, ds(k * JW + n * 512, 512)],
                                start=(c == 0), stop=(c == KCE - 1),
                            )
                for n in range(2):
                    if include_bias:
                        nc.vector.tensor_add(
                            pme2[n][:], pme2[n][:], b2_sb[:, ds(j * JW + n * 512, 512)])
                    nc.vector.tensor_scalar_mul(
                        er_bf[:, ds(j * JW + n * 512, 512)], pme2[n][:], wes[:])
                nc.scalar.dma_start(cin[:, ds(j * JW, JW)], er_bf[:, ds(j * JW, JW)])
                if j == 0:
                    pmh = ps.tile([B, 512], F32, name="pmh", tag="pm")
            nc.gpsimd.collective_compute(
                "AllReduce", ADD, replica_groups=[list(range(NCORES))],
                ins=[cin[:].opt()], outs=[wsout[:].opt()],
            )

            # ---- head GEMM1 (column shard) ----
            ws_sb = acts.tile([B, KD], BF16)
            wsT = acts.tile([P, KCE, B], BF16)
            nc.scalar.dma_start(ws_sb[:], wsout[:])
            for c in range(KCE):
                pt = ps.tile([P, B], BF16, name="pt", tag="pt", bufs=2)
                nc.tensor.transpose(pt[:], ws_sb[:, ts(c, P)], ident[:B, :B])
                nc.vector.tensor_copy(wsT[:, c, :], pt[:])
                nc.tensor.matmul(
                    pmh[:, :CH1C], wsT[:, c, :],
                    ch1_sb[:, ds(c * CH1C, CH1C)],
                    start=(c == 0), stop=(c == KCE - 1),
                )
            hh = acts.tile([B, CH1C], BF16)
            if include_bias:
                nc.vector.tensor_add(pmh[:, :CH1C], pmh[:, :CH1C], chb1_sb[:])
            nc.scalar.activation(hh[:], pmh[:, :CH1C], GELU)

            # ---- head GEMM2 (contraction shard): out_part = hh @ ch2_e ----
            hhT = acts.tile([P, 3, B], BF16)
            for c in range(3):
                pt = ps.tile([P, B], BF16, name="pt", tag="pt", bufs=2)
                nc.tensor.transpose(pt[:], hh[:, ts(c, P)], ident[:B, :B])
                nc.vector.tensor_copy(hhT[:, c, :], pt[:])
            outsb = acts.tile([B, C], F32)
            for nn in range(2):
                pmo = ps.tile([B, 512], F32, name="pmo", tag="pm")
                for c in range(3):
                    nc.tensor.matmul(
                        pmo[:, :500], hhT[:, c, :], ch2_sb[:, c, ds(nn * 500, 500)],
                        start=(c == 0), stop=(c == 2),
                    )
                nc.vector.tensor_copy(outsb[:, ds(nn * 500, 500)], pmo[:, :500])
            nc.scalar.dma_start(outp[:], outsb[:])

    nc.finalize()
    return nc


_NC_CACHE: dict = {}


def _get_nc(include_bias: bool) -> bass.Bass:
    if include_bias not in _NC_CACHE:
        _NC_CACHE[include_bias] = _build(include_bias)
    return _NC_CACHE[include_bias]


def _pack_w2(w2_e: np.ndarray) -> np.ndarray:
    """[3072, 3072] -> [NJ*W2SUB*128, W2K*1024] in (j, sub, k-interleaved) layout."""
    cols = w2_e.reshape(KD, NJ, JW)                       # split columns
    out = np.empty((NJ * W2SUB * P, W2K * JW), dtype=np.float32)
    for j in range(NJ):
        blk = cols[:, j, :].reshape(W2SUB, W2K, P, JW)    # [sub, k, p, jw]
        out[j * W2SUB * P:(j + 1) * W2SUB * P] = (
            blk.transpose(0, 2, 1, 3).reshape(W2SUB * P, W2K * JW))
    if W2_FP8:
        m = float(ml_dtypes.finfo(f8e3).max)
        return np.clip(out * W2_SCALE, -m, m).astype(f8e3)
    return out.astype(bf16)


def _pack_inputs(inputs: dict, include_bias: bool) -> list[dict]:
    f32 = np.float32
    x = np.ascontiguousarray(np.asarray(inputs["x"], dtype=f32))      # (64,32,384)
    expert_emb = np.asarray(inputs["expert_emb"], dtype=f32)          # (8,384)
    w1 = np.asarray(inputs["w1"])                                     # (8,3072,3072)
    w2 = np.asarray(inputs["w2"])
    sw_w1 = np.asarray(inputs["sw_w1"])                               # (12288,12288)
    sw_w2 = np.asarray(inputs["sw_w2"])                               # (12288,8)
    ch_w1 = np.asarray(inputs["ch_w1"])                               # (3072,3072)
    ch_w2 = np.asarray(inputs["ch_w2"])                               # (3072,1000)

    x2 = x.reshape(NTOK, D)
    xt_base = x2.T.reshape(3, P, NTOK).transpose(1, 0, 2)             # (128,3,2048)
    x2b = x2.astype(bf16)                                             # (2048,384)
    xf = x.reshape(B, ND)
    xft_p = np.ascontiguousarray(
        xf.T.reshape(KCS, P, B).transpose(1, 0, 2)).astype(bf16)      # (128,96,64)

    ch1_full = ch_w1.reshape(KD, E, CH1C)                             # col shards
    ch2_full = ch_w2.reshape(E, CH1C, C)                              # row shards

    in_maps = []
    for e in range(NCORES):
        emb_p = expert_emb[e].reshape(3, P).T                          # (128,3)
        xt_p = np.ascontiguousarray(
            np.concatenate([xt_base, emb_p[:, :, None]], axis=2), dtype=f32)
        sw1_e = sw_w1[:, e * SWC:(e + 1) * SWC]                        # (12288,1536)
        swd_p = np.ascontiguousarray(
            sw1_e.reshape(NSW, SWPACK, P, SWC).transpose(0, 2, 1, 3)
            .reshape(NSW * P, SWPACK * SWC)).astype(bf16)
        w1d_p = np.ascontiguousarray(
            np.asarray(w1[e], f32).reshape(NW1, W1PACK, P, KD)
            .transpose(0, 2, 1, 3).reshape(NW1 * P, W1PACK * KD)).astype(bf16)
        w2d_p = _pack_w2(np.asarray(w2[e], f32))
        sw2_e = np.ascontiguousarray(sw_w2[e * SWC:(e + 1) * SWC, :])  # (1536,8)
        sw2_p = np.ascontiguousarray(
            sw2_e.reshape(KCH, P, E).transpose(1, 0, 2)).astype(bf16)  # (128,12,8)
        ch1_p = np.ascontiguousarray(
            ch1_full[:, e, :].reshape(KCE, P, CH1C).transpose(1, 0, 2)
            .reshape(P, KCE * CH1C)).astype(bf16)                      # (128,24*384)
        ch2_p = np.ascontiguousarray(
            ch2_full[e].reshape(3, P, C).transpose(1, 0, 2)).astype(bf16)  # (128,3,1000)
        oh_p = np.zeros((B, E), dtype=f32)
        oh_p[:, e] = 1.0
        m = {
            "xt": xt_p, "x2b": x2b, "xft": xft_p,
            "swd": swd_p, "w1d": w1d_p, "w2d": w2d_p,
            "sw2": sw2_p, "ch1": ch1_p, "ch2": ch2_p, "oh": oh_p,
        }
        if include_bias:
            m["b1d"] = np.asarray(inputs["b1"][e], f32).reshape(1, KD)
            b2v = np.asarray(inputs["b2"][e], f32).reshape(1, KD)
            m["b2d"] = b2v * (W2_SCALE if W2_FP8 else 1.0)
            m["swb1d"] = np.asarray(
                inputs["sw_b1"], f32).reshape(1, ND)[:, e * SWC:(e + 1) * SWC]
            m["swb2d"] = np.asarray(inputs["sw_b2"], f32).reshape(1, E)
            m["chb1d"] = np.asarray(
                inputs["ch_b1"], f32).reshape(1, KD)[:, e * CH1C:(e + 1) * CH1C]
        in_maps.append(m)
    return in_maps


def _need_bias(inputs) -> bool:
    return any(
        float(np.abs(np.asarray(inputs[k])).max()) != 0.0
        for k in ("b1", "b2", "sw_b1", "sw_b2", "ch_b1")
    )


def run(inputs: dict, **run_kwargs):
    """Run on the 8 cores; returns (full_output, BassKernelResults)."""
    include_bias = _need_bias(inputs)
    nc = _get_nc(include_bias)
    in_maps = _pack_inputs(inputs, include_bias)
    res = run_bass_kernel_spmd(nc, in_maps, core_ids=list(range(NCORES)), **run_kwargs)
    out = np.zeros((B, C), dtype=np.float64)
    for e in range(NCORES):
        out += res.results[e]["outp"].astype(np.float64)
    out += np.asarray(inputs["ch_b2"], np.float64)
    return out.astype(np.float32), res


def kernel(**inputs) -> np.ndarray:
    out, _ = run(inputs)
    return out


# revision 9
# speedup vs baseline: 1.0595x; 1.0480x over previous
"""Expert-choice MoE kernel for 8 Trainium2 NeuronCores (expert-parallel).

Decomposition (core e handles expert e):
  - router logits x . emb_e computed in fp32 on PE; top-8 token indices per
    batch row via DVE max8/max_index; token gather via indirect DMA.
  - sum_weights GEMM1 column-sharded (each core owns 1536 columns of sw_w1);
    the tiny (8,64) partial logits are AllReduced, softmaxed locally.
  - expert MLP: GEMM1 (w1) in bf16; GEMM2 (w2) weights streamed as fp8e3m4
    scaled x128 (descale folded into the combine weights `wes`).
  - er * w[:, e] contributions AllReduced in bf16 in one shot (chunked
    ARs would just serialize on the CC engine behind the z-AR).
  - classification head sharded: GEMM1 column-shard (384 cols of ch_w1),
    GEMM2 contraction-shard (384 rows of ch_w2); per-core (64,1000) partials
    are summed on the host (+ ch_b2).

DMA ring assignment: sync(HWDGE/SP) ring carries ONLY the big weight
stream (sw1 -> w1 -> w2) in consumption order, packed into large
contiguous chunks (1.5MB sw / 1.5MB w1 / 1MB w2). Activations, inputs,
ch1/ch2 and collective staging ride the scalar(Act) ring; gathers and
collective triggers ride gpsimd (SWDGE).

PSUM plan (8 banks): tag "pm" bufs=6 + tag "pt" bufs=2 (transposes +
router). The "pm" ring allocation order is load-bearing (ring reuse must
only ever land on a dead tile): pms x3, pz, pme x6, pme2 x6, pmh, pmo x2.
"""

import numpy as np
import ml_dtypes

import concourse.bass as bass
from concourse import bacc
import concourse.mybir as mybir
import concourse.tile as tile
from concourse.bass import ts, ds
from concourse.bass_utils import run_bass_kernel_spmd
from concourse.masks import make_identity

B, N, D, E, K, C = 64, 32, 384, 8, 8, 1000
KD, ND = K * D, N * D          # 3072, 12288
P = 128
NTOK = B * N                   # 2048
SWC = ND // E                  # 1536 sum-weights columns per core
CH1C = KD // E                 # 384 head-GEMM1 columns per core
KCE = KD // P                  # 24 k-chunks, expert GEMMs
KCS = ND // P                  # 96 k-chunks, sum-weights GEMM1
KCH = SWC // P                 # 12 k-chunks, z GEMM
NCORES = 8

SWPACK = 4                     # k-chunks per sw1 DMA (1.5MB)
NSW = KCS // SWPACK            # 24 sw tiles
W1PACK = 2                     # k-chunks per w1 DMA (1.5MB)
NW1 = KCE // W1PACK            # 12 w1 tiles
W2SUB = 6                      # w2 sub-DMAs (full-width, 4 k-chunks each)
W2K = KCE // W2SUB             # 4 k-chunks per w2 sub-DMA

W2_FP8 = True                  # stream w2 as fp8e3m4 (scale 128)
W2_SCALE = 128.0

F32 = mybir.dt.float32
BF16 = mybir.dt.bfloat16
FP8E3 = mybir.dt.float8e3
U32 = mybir.dt.uint32
GELU = mybir.ActivationFunctionType.Gelu
EXP = mybir.ActivationFunctionType.Exp
X_AX = mybir.AxisListType.X
ADD = mybir.AluOpType.add
bf16 = ml_dtypes.bfloat16
f8e3 = ml_dtypes.float8_e3m4

W2DT = FP8E3 if W2_FP8 else BF16


def _build(include_bias: bool) -> bass.Bass:
    nc = bacc.Bacc("TRN2", num_devices=NCORES)

    # weight stream (sync ring), packed layouts produced by _pack_inputs
    swd = nc.dram_tensor("swd", [NSW * P, SWPACK * SWC], BF16, kind="ExternalInput")
    w1d = nc.dram_tensor("w1d", [NW1 * P, W1PACK * KD], BF16, kind="ExternalInput")
    w2d = nc.dram_tensor("w2d", [W2SUB * P, W2K * KD], W2DT, kind="ExternalInput")
    # everything else (scalar/gpsimd rings)
    xt = nc.dram_tensor("xt", [P, 3, NTOK + 1], F32, kind="ExternalInput")
    x2b = nc.dram_tensor("x2b", [NTOK, D], BF16, kind="ExternalInput")
    xft = nc.dram_tensor("xft", [P, KCS, B], BF16, kind="ExternalInput")
    sw2 = nc.dram_tensor("sw2", [P, KCH, E], BF16, kind="ExternalInput")
    ch1 = nc.dram_tensor("ch1", [P, KCE * CH1C], BF16, kind="ExternalInput")
    ch2 = nc.dram_tensor("ch2", [P, 3, C], BF16, kind="ExternalInput")
    oh = nc.dram_tensor("oh", [B, E], F32, kind="ExternalInput")
    if include_bias:
        b1d = nc.dram_tensor("b1d", [1, KD], F32, kind="ExternalInput")
        b2d = nc.dram_tensor("b2d", [1, KD], F32, kind="ExternalInput")  # pre-scaled
        swb1d = nc.dram_tensor("swb1d", [1, SWC], F32, kind="ExternalInput")
        swb2d = nc.dram_tensor("swb2d", [1, E], F32, kind="ExternalInput")
        chb1d = nc.dram_tensor("chb1d", [1, CH1C], F32, kind="ExternalInput")
    outp = nc.dram_tensor("outp", [B, C], F32, kind="ExternalOutput")

    with tile.TileContext(nc) as tc:
        with (
            tc.tile_pool(name="consts", bufs=1) as consts,
            tc.tile_pool(name="acts", bufs=1) as acts,
            tc.tile_pool(name="wpool", bufs=7) as wpool,
            tc.tile_pool(name="ps", bufs=6, space="PSUM") as ps,
            tc.tile_pool(name="dram", bufs=1, space="DRAM") as dram,
        ):
            # ---- constants / inputs on the scalar+gpsimd rings ----
            ident = consts.tile([P, P], BF16)
            make_identity(nc, ident[:])
            xft_sb = consts.tile([P, KCS, B], BF16)
            nc.scalar.dma_start(xft_sb[:], xft[:])
            xt_sb = acts.tile([P, 3, NTOK + 1], F32)
            nc.scalar.dma_start(xt_sb[:], xt[:])
            sw2_sb = consts.tile([P, KCH, E], BF16)
            nc.scalar.dma_start(sw2_sb[:], sw2[:])
            oh_sb = consts.tile([B, E], F32)
            nc.scalar.dma_start(oh_sb[:], oh[:])
            ch2_sb = consts.tile([P, 3, C], BF16)
            nc.scalar.dma_start(ch2_sb[:], ch2[:])
            if include_bias:
                b1_sb = consts.tile([B, KD], F32)
                nc.scalar.dma_start(b1_sb[:], b1d[0:1, :].to_broadcast([B, KD]))
                b2_sb = consts.tile([B, KD], F32)
                nc.scalar.dma_start(b2_sb[:], b2d[0:1, :].to_broadcast([B, KD]))
                swb1_sb = consts.tile([B, SWC], F32)
                nc.scalar.dma_start(swb1_sb[:], swb1d[0:1, :].to_broadcast([B, SWC]))
                swb2_sb = consts.tile([B, E], F32)
                nc.scalar.dma_start(swb2_sb[:], swb2d[0:1, :].to_broadcast([B, E]))
                chb1_sb = consts.tile([B, CH1C], F32)
                nc.scalar.dma_start(chb1_sb[:], chb1d[0:1, :].to_broadcast([B, CH1C]))

            pwarm = ps.tile([P, B], BF16, name="pwarm", tag="pt", bufs=2)
            nc.tensor.transpose(pwarm[:32, :32], ident[:32, :32], ident[:32, :32])

            # ---- sum-weights GEMM1: stream sw1, accumulate 3 banks ----
            pms = [ps.tile([B, 512], F32, name=f"pms{n}", tag="pm")
                   for n in range(3)]
            for t in range(NSW):
                wt = wpool.tile([P, SWPACK * SWC], BF16, name="wt", tag="wt")
                nc.sync.dma_start(wt[:], swd[ts(t, P), :])
                for k in range(SWPACK):
                    c = t * SWPACK + k
                    for n in range(3):
                        nc.tensor.matmul(
                            pms[n][:], xft_sb[:, c, :],
                            wt[:, ds(k * SWC + n * 512, 512)],
                            start=(c == 0), stop=(c == KCS - 1),
                        )
                if t == 1:
                    # ---- router (after xt lands): logits = x @ emb_e ----
                    lg_flat = acts.tile([1, NTOK], F32)
                    for nt in range(4):
                        pr = ps.tile([B, 512], F32, name="pr", tag="pt", bufs=2)
                        for cc in range(3):
                            nc.tensor.matmul(
                                pr[:1, :], xt_sb[:, cc, NTOK : NTOK + 1],
                                xt_sb[:, cc, ts(nt, 512)],
                                start=(cc == 0), stop=(cc == 2),
                            )
                        nc.vector.tensor_copy(lg_flat[:, ts(nt, 512)], pr[:1, :])
                    lg_dram = dram.tile([1, NTOK], F32)
                    nc.scalar.dma_start(lg_dram[:], lg_flat[:])
                    lg_bn = acts.tile([B, N], F32)
                    nc.scalar.dma_start(
                        lg_bn[:], lg_dram[:].rearrange("x (b n) -> (x b) n", b=B))
                if t == 2:
                    # ---- top-8 per row + token gather ----
                    vals8 = acts.tile([B, 8], F32)
                    idx8 = acts.tile([B, 8], U32)
                    nc.vector.max(out=vals8[:], in_=lg_bn[:])
                    nc.vector.max_index(out=idx8[:], in_max=vals8[:], in_values=lg_bn[:])
                    base = acts.tile([B, 1], U32)
                    nc.gpsimd.iota(base[:], pattern=[[0, 1]], base=0, channel_multiplier=N)
                    off = acts.tile([B, 8], U32)
                    nc.vector.tensor_tensor(
                        out=off[:], in0=idx8[:], in1=base[:].to_broadcast([B, 8]), op=ADD)
                    sel = acts.tile([B, K, D], BF16)
                    for k in range(K):
                        nc.gpsimd.indirect_dma_start(
                            out=sel[:, k, :], out_offset=None,
                            in_=x2b[:],
                            in_offset=bass.IndirectOffsetOnAxis(ap=off[:, k : k + 1], axis=0),
                        )
                if t == 3:
                    # selT chunks [128, 24, 64] for expert GEMM1 stationaries
                    sel_flat = sel[:].rearrange("b k d -> b (k d)")
                    selT = acts.tile([P, KCE, B], BF16)
                    for c in range(KCE):
                        pt = ps.tile([P, B], BF16, name="pt", tag="pt", bufs=2)
                        nc.tensor.transpose(pt[:], sel_flat[:, ts(c, P)], ident[:B, :B])
                        nc.vector.tensor_copy(selT[:, c, :], pt[:])

            # ---- z = h1 @ sw2 partials, AllReduce (tiny, fp32) ----
            h1 = acts.tile([B, SWC], BF16)
            for n in range(3):
                if include_bias:
                    nc.vector.tensor_add(pms[n][:], pms[n][:], swb1_sb[:, ts(n, 512)])
                nc.scalar.activation(h1[:, ts(n, 512)], pms[n][:], GELU)
            h1T = acts.tile([P, KCH, B], BF16)
            for c in range(KCH):
                pt = ps.tile([P, B], BF16, name="pt", tag="pt", bufs=2)
                nc.tensor.transpose(pt[:], h1[:, ts(c, P)], ident[:B, :B])
                nc.vector.tensor_copy(h1T[:, c, :], pt[:])
            pz = ps.tile([B, 512], F32, name="pz", tag="pm")
            for c in range(KCH):
                nc.tensor.matmul(
                    pz[:E, :B], sw2_sb[:, c, :], h1T[:, c, :],
                    start=(c == 0), stop=(c == KCH - 1),
                )
            zT_sb = acts.tile([E, B], F32)
            nc.vector.tensor_copy(zT_sb[:], pz[:E, :B])
            zin = dram.tile([E, B], F32)
            zout = dram.tile([E, B], F32)
            nc.scalar.dma_start(zin[:], zT_sb[:])
            nc.gpsimd.collective_compute(
                "AllReduce", ADD, replica_groups=[list(range(NCORES))],
                ins=[zin[:].opt()], outs=[zout[:].opt()],
            )

            # ---- expert GEMM1: h = gelu(selT.T @ w1_e) ----
            h = acts.tile([B, KD], BF16)
            pme = [ps.tile([B, 512], F32, name=f"pme{n}", tag="pm")
                   for n in range(6)]
            for t in range(NW1):
                wt = wpool.tile([P, W1PACK * KD], BF16, name="wt", tag="wt")
                nc.sync.dma_start(wt[:], w1d[ts(t, P), :])
                if t == 1:
                    # prefetch ch1 on the scalar ring (needed from the head on)
                    ch1_sb = consts.tile([P, KCE * CH1C], BF16)
                    nc.scalar.dma_start(ch1_sb[:], ch1[:])
                for k in range(W1PACK):
                    c = t * W1PACK + k
                    for n in range(6):
                        nc.tensor.matmul(
                            pme[n][:], selT[:, c, :],
                            wt[:, ds(k * KD + n * 512, 512)],
                            start=(c == 0), stop=(c == KCE - 1),
                        )
            last_gelu = None
            for n in range(6):
                if include_bias:
                    nc.vector.tensor_add(pme[n][:], pme[n][:], b1_sb[:, ts(n, 512)])
                last_gelu = nc.scalar.activation(h[:, ts(n, 512)], pme[n][:], GELU)
            hT = acts.tile([P, KCE, B], BF16)
            last_htc = None
            for c in range(KCE):
                pt = ps.tile([P, B], BF16, name="pt", tag="pt", bufs=2)
                nc.tensor.transpose(pt[:], h[:, ts(c, P)], ident[:B, :B])
                last_htc = nc.vector.tensor_copy(hT[:, c, :], pt[:])

            # ---- softmax over experts; wes = w[:, e] / W2_SCALE.  Emitted
            # after the h gelus/hT copies so no z-dependent op can park the
            # ACT or DVE FIFOs while GEMM1 output processing is pending. ----
            zb = acts.tile([B, E], F32)
            nc.gpsimd.dma_start(zb[:], zout[:].rearrange("e b -> b e"))
            if include_bias:
                nc.vector.tensor_add(zb[:], zb[:], swb2_sb[:])
            mx = acts.tile([B, 1], F32)
            mx_i = nc.vector.reduce_max(mx[:], zb[:], axis=X_AX)
            tile.add_dep_helper(mx_i.ins, last_htc.ins, sync=False,
                                reason="softmax after hT copies on DVE")
            nmx = acts.tile([B, 1], F32)
            nc.vector.tensor_scalar_mul(nmx[:], mx[:], -1.0)
            exps = acts.tile([B, E], F32)
            exp_i = nc.scalar.activation(exps[:], zb[:], EXP, bias=nmx[:])
            tile.add_dep_helper(exp_i.ins, last_gelu.ins, sync=False,
                                reason="Exp after expert gelus on ACT")
            sm = acts.tile([B, 1], F32)
            nc.vector.reduce_sum(sm[:], exps[:], axis=X_AX)
            rs = acts.tile([B, 1], F32)
            nc.vector.reciprocal(rs[:], sm[:])
            wv = acts.tile([B, E], F32)
            nc.vector.tensor_scalar_mul(wv[:], exps[:], rs[:])
            t8 = acts.tile([B, E], F32)
            nc.vector.tensor_mul(out=t8[:], in0=wv[:], in1=oh_sb[:])
            wes = acts.tile([B, 1], F32)
            nc.vector.reduce_sum(wes[:], t8[:], axis=X_AX)
            if W2_FP8:
                nc.vector.tensor_scalar_mul(wes[:], wes[:], 1.0 / W2_SCALE)

            # ---- expert GEMM2, single pass over all 3072 columns with 6
            # PSUM banks (same stationary amortization as GEMM1); ONE bf16
            # AllReduce for the whole [B, KD] er (chunked ARs would only
            # serialize on the CC engine behind the z-AR anyway, and each
            # extra AR pays its own mesh latency). ----
            cin = dram.tile([B, KD], BF16, name="cin")
            wsout = dram.tile([B, KD], BF16, name="wsout")
            er_bf = acts.tile([B, KD], BF16)
            pme2 = [ps.tile([B, 512], F32, name=f"pme2{n}", tag="pm")
                    for n in range(6)]
            for s in range(W2SUB):
                wt = wpool.tile([P, W2K * KD], W2DT, name="wt", tag="wt")
                nc.sync.dma_start(wt[:], w2d[ts(s, P), :])
                for k in range(W2K):
                    c = s * W2K + k
                    for n in range(6):
                        nc.tensor.matmul(
                            pme2[n][:], hT[:, c, :],
                            wt[:, ds(k * KD + n * 512, 512)],
                            start=(c == 0), stop=(c == KCE - 1),
                        )
            for n in range(6):
                if include_bias:
                    nc.vector.tensor_add(
                        pme2[n][:], pme2[n][:], b2_sb[:, ts(n, 512)])
                nc.vector.tensor_scalar_mul(
                    er_bf[:, ts(n, 512)], pme2[n][:], wes[:])
            nc.scalar.dma_start(cin[:], er_bf[:])
            pmh = ps.tile([B, 512], F32, name="pmh", tag="pm")
            nc.gpsimd.collective_compute(
                "AllReduce", ADD, replica_groups=[list(range(NCORES))],
                ins=[cin[:].opt()], outs=[wsout[:].opt()],
            )

            # ---- head GEMM1 (column shard), ws loaded in 3 slices so the
            # transposes/matmuls pipeline with the readback ----
            ws_sb = acts.tile([B, KD], BF16)
            wsT = acts.tile([P, KCE, B], BF16)
            for j in range(3):
                nc.scalar.dma_start(
                    ws_sb[:, ds(j * KD // 3, KD // 3)],
                    wsout[:, ds(j * KD // 3, KD // 3)])
                for tt in range(KCE // 3):
                    c = j * (KCE // 3) + tt
                    pt = ps.tile([P, B], BF16, name="pt", tag="pt", bufs=2)
                    nc.tensor.transpose(pt[:], ws_sb[:, ts(c, P)], ident[:B, :B])
                    nc.vector.tensor_copy(wsT[:, c, :], pt[:])
                    nc.tensor.matmul(
                        pmh[:, :CH1C], wsT[:, c, :],
                        ch1_sb[:, ds(c * CH1C, CH1C)],
                        start=(c == 0), stop=(c == KCE - 1),
                    )
            hh = acts.tile([B, CH1C], BF16)
            if include_bias:
                nc.vector.tensor_add(pmh[:, :CH1C], pmh[:, :CH1C], chb1_sb[:])
            nc.scalar.activation(hh[:], pmh[:, :CH1C], GELU)

            # ---- head GEMM2 (contraction shard): out_part = hh @ ch2_e ----
            hhT = acts.tile([P, 3, B], BF16)
            for c in range(3):
                pt = ps.tile([P, B], BF16, name="pt", tag="pt", bufs=2)
                nc.tensor.transpose(pt[:], hh[:, ts(c, P)], ident[:B, :B])
                nc.vector.tensor_copy(hhT[:, c, :], pt[:])
            outsb = acts.tile([B, C], F32)
            for nn in range(2):
                pmo = ps.tile([B, 512], F32, name="pmo", tag="pm")
                for c in range(3):
                    nc.tensor.matmul(
                        pmo[:, :500], hhT[:, c, :], ch2_sb[:, c, ds(nn * 500, 500)],
                        start=(c == 0), stop=(c == 2),
                    )
                nc.vector.tensor_copy(outsb[:, ds(nn * 500, 500)], pmo[:, :500])
            nc.scalar.dma_start(outp[:], outsb[:])

    nc.finalize()
    return nc


_NC_CACHE: dict = {}


def _get_nc(include_bias: bool) -> bass.Bass:
    if include_bias not in _NC_CACHE:
        _NC_CACHE[include_bias] = _build(include_bias)
    return _NC_CACHE[include_bias]


def _pack_w2(w2_e: np.ndarray) -> np.ndarray:
    """[3072, 3072] -> [W2SUB*128, W2K*3072], k-chunks packed per sub-DMA."""
    out = (w2_e.reshape(W2SUB, W2K, P, KD).transpose(0, 2, 1, 3)
           .reshape(W2SUB * P, W2K * KD))
    if W2_FP8:
        m = float(ml_dtypes.finfo(f8e3).max)
        return np.clip(out * W2_SCALE, -m, m).astype(f8e3)
    return np.ascontiguousarray(out).astype(bf16)


def _pack_inputs(inputs: dict, include_bias: bool) -> list[dict]:
    f32 = np.float32
    x = np.ascontiguousarray(np.asarray(inputs["x"], dtype=f32))      # (64,32,384)
    expert_emb = np.asarray(inputs["expert_emb"], dtype=f32)          # (8,384)
    w1 = np.asarray(inputs["w1"])                                     # (8,3072,3072)
    w2 = np.asarray(inputs["w2"])
    sw_w1 = np.asarray(inputs["sw_w1"])                               # (12288,12288)
    sw_w2 = np.asarray(inputs["sw_w2"])                               # (12288,8)
    ch_w1 = np.asarray(inputs["ch_w1"])                               # (3072,3072)
    ch_w2 = np.asarray(inputs["ch_w2"])                               # (3072,1000)

    x2 = x.reshape(NTOK, D)
    xt_base = x2.T.reshape(3, P, NTOK).transpose(1, 0, 2)             # (128,3,2048)
    x2b = x2.astype(bf16)                                             # (2048,384)
    xf = x.reshape(B, ND)
    xft_p = np.ascontiguousarray(
        xf.T.reshape(KCS, P, B).transpose(1, 0, 2)).astype(bf16)      # (128,96,64)

    ch1_full = ch_w1.reshape(KD, E, CH1C)                             # col shards
    ch2_full = ch_w2.reshape(E, CH1C, C)                              # row shards

    in_maps = []
    for e in range(NCORES):
        emb_p = expert_emb[e].reshape(3, P).T                          # (128,3)
        xt_p = np.ascontiguousarray(
            np.concatenate([xt_base, emb_p[:, :, None]], axis=2), dtype=f32)
        sw1_e = sw_w1[:, e * SWC:(e + 1) * SWC]                        # (12288,1536)
        swd_p = np.ascontiguousarray(
            sw1_e.reshape(NSW, SWPACK, P, SWC).transpose(0, 2, 1, 3)
            .reshape(NSW * P, SWPACK * SWC)).astype(bf16)
        w1d_p = np.ascontiguousarray(
            np.asarray(w1[e], f32).reshape(NW1, W1PACK, P, KD)
            .transpose(0, 2, 1, 3).reshape(NW1 * P, W1PACK * KD)).astype(bf16)
        w2d_p = _pack_w2(np.asarray(w2[e], f32))
        sw2_e = np.ascontiguousarray(sw_w2[e * SWC:(e + 1) * SWC, :])  # (1536,8)
        sw2_p = np.ascontiguousarray(
            sw2_e.reshape(KCH, P, E).transpose(1, 0, 2)).astype(bf16)  # (128,12,8)
        ch1_p = np.ascontiguousarray(
            ch1_full[:, e, :].reshape(KCE, P, CH1C).transpose(1, 0, 2)
            .reshape(P, KCE * CH1C)).astype(bf16)                      # (128,24*384)
        ch2_p = np.ascontiguousarray(
            ch2_full[e].reshape(3, P, C).transpose(1, 0, 2)).astype(bf16)  # (128,3,1000)
        oh_p = np.zeros((B, E), dtype=f32)
        oh_p[:, e] = 1.0
        m = {
            "xt": xt_p, "x2b": x2b, "xft": xft_p,
            "swd": swd_p, "w1d": w1d_p, "w2d": w2d_p,
            "sw2": sw2_p, "ch1": ch1_p, "ch2": ch2_p, "oh": oh_p,
        }
        if include_bias:
            m["b1d"] = np.asarray(inputs["b1"][e], f32).reshape(1, KD)
            b2v = np.asarray(inputs["b2"][e], f32).reshape(1, KD)
            m["b2d"] = b2v * (W2_SCALE if W2_FP8 else 1.0)
            m["swb1d"] = np.asarray(
                inputs["sw_b1"], f32).reshape(1, ND)[:, e * SWC:(e + 1) * SWC]
            m["swb2d"] = np.asarray(inputs["sw_b2"], f32).reshape(1, E)
            m["chb1d"] = np.asarray(
                inputs["ch_b1"], f32).reshape(1, KD)[:, e * CH1C:(e + 1) * CH1C]
        in_maps.append(m)
    return in_maps


def _need_bias(inputs) -> bool:
    return any(
        float(np.abs(np.asarray(inputs[k])).max()) != 0.0
        for k in ("b1", "b2", "sw_b1", "sw_b2", "ch_b1")
    )


def run(inputs: dict, **run_kwargs):
    """Run on the 8 cores; returns (full_output, BassKernelResults)."""
    include_bias = _need_bias(inputs)
    nc = _get_nc(include_bias)
    in_maps = _pack_inputs(inputs, include_bias)
    res = run_bass_kernel_spmd(nc, in_maps, core_ids=list(range(NCORES)), **run_kwargs)
    out = np.zeros((B, C), dtype=np.float64)
    for e in range(NCORES):
        out += res.results[e]["outp"].astype(np.float64)
    out += np.asarray(inputs["ch_b2"], np.float64)
    return out.astype(np.float32), res


def kernel(**inputs) -> np.ndarray:
    out, _ = run(inputs)
    return out
